# revision 1
# baseline (speedup 1.0000x reference)
"""BatchBlur_SV (19x19 box-sum, reflect pad) on 8 TRN2 NeuronCores.

Strategy
--------
Data parallel over batch: 16 images -> 2 per core (6 [1024,1024] planes).

The 19x19 box sum is separable into an H-pass and a W-pass. Each pass is
computed on the TensorEngine as a set of banded-ones matmuls with the
*data block as the stationary operand*:

    out[m, n] = sum_k lhsT[k, m] * band[k, n]

With lhsT = X[h-block i, w-chunk j] (contraction k = h) and the moving
operand a constant band matrix band_i[k, n] (ones where |h_out - h| <= 9,
reflection folded into the edge blocks), the output lands as
Y^T[w-chunk, h_out] in PSUM. Running the identical pass again on Y^T
contracts w and lands Z[h-chunk, w_out] - natural layout. No transposes,
no halo DMA. Adjacent blocks' output windows overlap by 18 columns;
PSUM's per-element has_written bit (start=True on the first matmul in a
bank marks the whole 2KB zero-region) makes later matmuls overwrite
fresh columns and accumulate on overlapped ones.

Compute dtype is fp16 (1 cyc/row on PE vs 4 for fp32); accumulation is
fp32 in PSUM. I/O is fp16 (host casts), halving HBM traffic.
"""

import sys

if "/opt/trn_rl_repo" not in sys.path:
    sys.path.insert(0, "/opt/trn_rl_repo")

import numpy as np

L = 19
R = L // 2  # 9
H = W = 1024
BK = 128  # block size (partitions)
NB = H // BK  # 8 blocks per axis
NCORES = 8
NPLANES = 6  # (16 batches / 8 cores) * 3 channels
BANDW = BK + 2 * R  # 146: max output-window width of one block
PSUM_BANK = 512  # fp32 elements per PSUM bank per partition

_cache = {}


def _reflect(t):
    if t < 0:
        return -t
    if t > H - 1:
        return 2 * (H - 1) - t
    return t


def _make_bands():
    """band_i[k, c]: contribution count of block-local row k (global
    h = 128i + k) to output col (win_start_i + c). Reflection folds into
    blocks 0 and NB-1. Entries are 0/1/2 - exact in fp16."""
    bands = np.zeros((NB, BK, BANDW), dtype=np.float16)
    wins = []
    for i in range(NB):
        n0 = max(0, BK * i - R)
        n1 = min(H, BK * i + BK + R)
        wins.append((n0, n1))
        for o in range(n0, n1):
            for j in range(L):
                src = _reflect(o - R + j)
                if BK * i <= src < BK * i + BK:
                    bands[i, src - BK * i, o - n0] += 1.0
    return bands, wins


def _piece_table(wins, sim_safe):
    """Per contraction-block i: ordered (col_a, col_b, start, stop, bank).

    Cut points: PSUM bank boundaries always; with sim_safe additionally
    the boundary between the previous block's window end (accumulate
    region) and the fresh region, so every matmul region is uniformly
    fresh or uniformly accumulating (CoreSim asserts this; HW is
    per-element and doesn't need it).
    """
    per_bank = {}
    table = {i: [] for i in range(NB)}
    for i in range(NB):
        n0, n1 = wins[i]
        cuts = {n0, n1}
        cuts.update(c for c in range(PSUM_BANK, H, PSUM_BANK) if n0 < c < n1)
        if sim_safe and i > 0:
            prev_end = wins[i - 1][1]
            if n0 < prev_end < n1:
                cuts.add(prev_end)
        cuts = sorted(cuts)
        for a, b in zip(cuts[:-1], cuts[1:]):
            bank = a // PSUM_BANK
            per_bank.setdefault(bank, []).append((i, a, b))
    flags = {}
    for bank, ps in per_bank.items():
        for idx, p in enumerate(ps):
            flags[p] = (idx == 0, idx == len(ps) - 1)
    for bank, ps in per_bank.items():
        for i, a, b in ps:
            st, sp = flags[(i, a, b)]
            table[i].append((a, b, st, sp, bank))
    for i in range(NB):
        table[i].sort()
    return table


def _build(sim_safe=False):
    import concourse.bacc as bacc
    import concourse.bass as bass
    import concourse.mybir as mybir
    import concourse.tile as tile
    from bass_rust import add_dep_helper

    f16 = mybir.dt.float16
    f32 = mybir.dt.float32

    bands_np, wins = _make_bands()
    pieces = _piece_table(wins, sim_safe)

    nc = bacc.Bacc(
        "TRN2", target_bir_lowering=False, debug=False, num_devices=NCORES
    )
    x_ext = nc.dram_tensor("x", [NPLANES, NB, BK, W], f16, kind="ExternalInput")
    b_ext = nc.dram_tensor("bands", [NB, BK, BANDW], f16, kind="ExternalInput")
    o_ext = nc.dram_tensor("out", [NPLANES, NB, BK, W], f16, kind="ExternalOutput")

    copy_ctr = [0]

    def box_pass(tc, src_t, dst_t, bands_t, pspool):
        # src_t[p, t, f] = plane(axisA = BK*t + p, axisB = f)
        # dst_t[p, t, f] = out(axisB = BK*t + p, axisA_out = f)  (flipped)
        for j in range(NB):
            ps = pspool.tile([BK, H], f32, tag="ps")
            bank_start = {}
            for i in range(NB):
                lhsT = src_t[:, i, BK * j : BK * (j + 1)]
                n0 = wins[i][0]
                for a, b, st, sp, bank in pieces[i]:
                    inst = nc.tensor.matmul(
                        ps[:, a:b],
                        lhsT,
                        bands_t[:, i, a - n0 : b - n0],
                        start=st,
                        stop=sp,
                    )
                    if st:
                        bank_start[bank] = inst
                    else:
                        # ensure every accumulating piece is scheduled
                        # after the matmul that marked its bank's
                        # zero-region (same engine: order-only dep)
                        add_dep_helper(inst.ins, bank_start[bank].ins, False)
            # PSUM fp32 -> SBUF fp16 cast copy. The last two strips gate
            # the next pass's first matmuls, so split them across both
            # engines to halve their latency; alternate DVE/ACT otherwise.
            if j >= NB - 2:
                nc.vector.tensor_copy(dst_t[:, j, :PSUM_BANK], ps[:, :PSUM_BANK])
                nc.scalar.copy(dst_t[:, j, PSUM_BANK:], ps[:, PSUM_BANK:])
            elif copy_ctr[0] % 2 == 0:
                nc.vector.tensor_copy(dst_t[:, j, :], ps[:])
            else:
                nc.scalar.copy(dst_t[:, j, :], ps[:])
            copy_ctr[0] += 1

    with tile.TileContext(nc) as tc:
        with (
            tc.tile_pool(name="const", bufs=1) as cpool,
            tc.tile_pool(name="xp", bufs=3) as xpool,
            tc.tile_pool(name="yp", bufs=3) as ypool,
            tc.tile_pool(name="zp", bufs=3) as zpool,
            tc.tile_pool(name="ps", bufs=4, space=bass.MemorySpace.PSUM) as pspool,
        ):
            # bands on the scalar HWDGE ring so they don't delay the
            # plane-0 load on the sync ring
            bands_t = cpool.tile([BK, NB, BANDW], f16)
            nc.scalar.dma_start(out=bands_t[:], in_=b_ext.rearrange("i p c -> p i c"))

            def load_plane(pl):
                x_t = xpool.tile([BK, NB, W], f16, tag="x")
                xv = x_ext[pl].rearrange("t p f -> p t f")
                if pl == 0:
                    # column-chunked first load: group j only needs cols
                    # [128j, 128j+128), so compute starts once the first
                    # small chunk lands
                    for c0, c1 in ((0, 128), (128, 384), (384, 704), (704, 1024)):
                        cs = slice(c0, c1)
                        nc.sync.dma_start(out=x_t[:, :, cs], in_=xv[:, :, cs])
                else:
                    nc.sync.dma_start(out=x_t[:], in_=xv)
                return x_t

            def store_plane(pl, z_t):
                ov = o_ext[pl].rearrange("t p f -> p t f")
                if pl < NPLANES - 1:
                    nc.scalar.dma_start(out=ov[:], in_=z_t[:])
                else:
                    # last plane: quarter stores so the final drain is short
                    for h in range(4):
                        hs = slice(2 * h, 2 * (h + 1))
                        nc.scalar.dma_start(out=ov[:, hs, :], in_=z_t[:, hs, :])

            for pl in range(NPLANES):
                x_t = load_plane(pl)
                y_t = ypool.tile([BK, NB, W], f16, tag="y")
                box_pass(tc, x_t, y_t, bands_t, pspool)
                z_t = zpool.tile([BK, NB, W], f16, tag="z")
                box_pass(tc, y_t, z_t, bands_t, pspool)
                store_plane(pl, z_t)

    nc.compile()
    return nc, bands_np


def _get_compiled(sim_safe=False):
    key = ("nc", sim_safe)
    if key not in _cache:
        _cache[key] = _build(sim_safe)
    return _cache[key]


def _run(input, trace=False, sim_safe=False):
    from concourse.bass_utils import run_bass_kernel_spmd

    nc, bands_np = _get_compiled(sim_safe)

    x = np.ascontiguousarray(input)
    assert x.shape == (16, 3, H, W), x.shape
    # [16,3,H,W] -> per-core [NPLANES, NB, BK, W] fp16 shards
    shards = x.reshape(NCORES, NPLANES, NB, BK, W).astype(np.float16)
    in_maps = [{"x": shards[c], "bands": bands_np} for c in range(NCORES)]

    res = run_bass_kernel_spmd(nc, in_maps, list(range(NCORES)), trace=trace)
    outs = np.stack([r["out"] for r in res.results])  # [8, 6, 8, 128, 1024] f16
    full = outs.reshape(16, 3, H, W).astype(np.float32)
    return full, res


def kernel(input):
    full, _ = _run(input)
    return full


def _make_exec_fn(nc, n_inputs):
    """Build a staged single-execution jitted fn for a compiled Bacc
    program, mirroring bass2jax.run_bass_via_pjrt's packaging."""
    import jax
    from jax.sharding import Mesh, PartitionSpec

    try:
        from jax.experimental.shard_map import shard_map
    except ImportError:
        from jax import shard_map

    from concourse import mybir
    from concourse.bass2jax import (
        _bass_exec_p,
        install_neuronx_cc_hook,
        partition_id_tensor,
    )

    install_neuronx_cc_hook()
    partition_name = nc.partition_id_tensor.name if nc.partition_id_tensor else None
    in_names, out_names, out_avals = [], [], []
    for alloc in nc.m.functions[0].allocations:
        if not isinstance(alloc, mybir.MemoryLocationSet):
            continue
        name = alloc.memorylocations[0].name
        if alloc.kind == "ExternalInput":
            if name != partition_name:
                in_names.append(name)
        elif alloc.kind == "ExternalOutput":
            shape = tuple(alloc.tensor_shape)
            np_dt = mybir.dt.np(alloc.dtype)
            out_avals.append(jax.core.ShapedArray(shape, np_dt))
            out_names.append(name)
    assert len(in_names) == n_inputs, (in_names, n_inputs)
    bind_in_names = tuple(
        in_names + out_names + ([partition_name] if partition_name else [])
    )

    devices = jax.devices()[:NCORES]
    mesh = Mesh(np.asarray(devices), ("core",))
    spec = PartitionSpec("core")

    def _body(*args):
        outs = _bass_exec_p.bind(
            *args,
            partition_id_tensor(),
            out_avals=tuple(out_avals),
            in_names=bind_in_names,
            out_names=tuple(out_names),
            lowering_input_output_aliases=(),
            sim_require_finite=True,
            sim_require_nnan=True,
            nc=nc,
        )
        return tuple(outs)

    nargs = len(in_names) + len(out_names)
    fn = jax.jit(
        shard_map(
            _body,
            mesh=mesh,
            in_specs=(spec,) * nargs,
            out_specs=(spec,) * len(out_names),
            check_rep=False,
        )
    )
    return fn, mesh, spec, out_avals, out_names


def _build_null():
    """Minimal kernel (one small DMA round-trip) - dispatch-overhead baseline."""
    import concourse.bacc as bacc
    import concourse.mybir as mybir
    import concourse.tile as tile

    f16 = mybir.dt.float16
    nc = bacc.Bacc("TRN2", target_bir_lowering=False, debug=False, num_devices=NCORES)
    x_ext = nc.dram_tensor("x", [BK, BK], f16, kind="ExternalInput")
    o_ext = nc.dram_tensor("out", [BK, BK], f16, kind="ExternalOutput")
    with tile.TileContext(nc) as tc:
        with tc.tile_pool(name="p", bufs=1) as pool:
            t = pool.tile([BK, BK], f16)
            nc.sync.dma_start(out=t[:], in_=x_ext[:])
            nc.sync.dma_start(out=o_ext[:], in_=t[:])
    nc.compile()
    return nc


def _bench(input, repeats=30):
    """Time staged single executions of the full kernel and of a null
    kernel; HW exec estimate = min(full) - min(null). Returns dict."""
    import time

    import jax
    from jax.sharding import NamedSharding

    nc, bands_np = _get_compiled()
    nc_null = _cache.setdefault("null", _build_null())

    x = np.ascontiguousarray(input)
    shards = x.reshape(NCORES, NPLANES, NB, BK, W).astype(np.float16)

    fn, mesh, spec, _, _ = _make_exec_fn(nc, 2)
    fn_null, _, _, _, _ = _make_exec_fn(nc_null, 1)

    sh = NamedSharding(mesh, spec)
    xg = jax.device_put(shards.reshape(NCORES * NPLANES, NB, BK, W), sh)
    bg = jax.device_put(
        np.broadcast_to(bands_np, (NCORES,) + bands_np.shape).reshape(
            NCORES * NB, BK, BANDW
        ),
        sh,
    )
    zg = jax.device_put(np.zeros((NCORES * NPLANES, NB, BK, W), np.float16), sh)
    xn = jax.device_put(np.zeros((NCORES * BK, BK), np.float16), sh)
    zn = jax.device_put(np.zeros((NCORES * BK, BK), np.float16), sh)
    jax.block_until_ready((xg, bg, zg, xn, zn))

    jax.block_until_ready(fn(xg, bg, zg))  # compile+warm
    jax.block_until_ready(fn_null(xn, zn))

    t_full, t_null = [], []
    for _ in range(repeats):
        t0 = time.perf_counter()
        jax.block_until_ready(fn(xg, bg, zg))
        t_full.append(time.perf_counter() - t0)
        t0 = time.perf_counter()
        jax.block_until_ready(fn_null(xn, zn))
        t_null.append(time.perf_counter() - t0)
    est_ns = (min(t_full) - min(t_null)) * 1e9
    return {
        "hw_exec_ns_est": est_ns,
        "min_full_s": min(t_full),
        "min_null_s": min(t_null),
        "full_s": t_full,
        "null_s": t_null,
    }

